# revision 8
# baseline (speedup 1.0000x reference)
"""Bass/Trainium2 kernel for nn_BilinearMixture (gnn_message_passing).

out = softmax(einsum('ed,kd,ed->ek', u_feat[u_idx], basis_weights, v_feat[v_idx])
              @ weights_scalars + user_bias[u_idx] + item_bias[v_idx])

Strategy (8 NeuronCores, SPMD, edges sharded across cores):
  * Fold the K x D x C basis projection into one M = basis_weights.T @ weights_scalars
    [D, C] matrix (computed on device), so per edge: logits = (u .* v) @ M.
  * Biases are folded into the gather via row augmentation:
      u' = [u, user_bias, 1s, 0...],  v' = [v, 1s, item_bias, 0...],  M' = [M; I; I; 0]
    so logits = sum_d u'_d v'_d M'[d, :] needs no separate bias gathers.
  * Tables are converted to fp16 and padded to 128-element (256 B) rows; edge rows are
    fetched with dma_gather(transpose=True), which lands gathered rows with the feature
    dim on SBUF partitions -- no on-chip transposes needed before the PE matmul.
  * dma_gather indices are int16 (<= 32767), so tables are split into 32768-row
    segments and edges are bucketed host-side by (u_seg, v_seg); each 4096-edge tile
    draws from a single bucket. The bucket permutation is undone host-side.
  * Softmax: logits are tiny (|logit| < ~2), so exp/sum without max subtraction.
"""

import numpy as np

NU, NI, D, E0, K, C = 100000, 50000, 64, 2000000, 3, 5
CORES = 8
SEG = 32768
EL = 128            # fp16 elements per padded table row (256 B)
T = 4096            # edges per tile
CH = T // 128       # matmul chunks (and per-partition edge count) per tile
USEG = (NU + SEG - 1) // SEG   # 4
VSEG = (NI + SEG - 1) // SEG   # 2

_PROG_CACHE = {}


def _build_program(schedule):
    import concourse.bacc as bacc
    import concourse.mybir as mybir
    import concourse.tile as tile

    NT = len(schedule)
    f16, f32, i16 = mybir.dt.float16, mybir.dt.float32, mybir.dt.int16

    nc = bacc.Bacc("TRN2", target_bir_lowering=False, debug=False, num_devices=CORES)
    u16 = nc.dram_tensor("u16", [USEG * SEG, EL], f16, kind="ExternalInput")
    v16 = nc.dram_tensor("v16", [VSEG * SEG, EL], f16, kind="ExternalInput")
    uidx = nc.dram_tensor("uidx", [NT, 128, T // 16], i16, kind="ExternalInput")
    vidx = nc.dram_tensor("vidx", [NT, 128, T // 16], i16, kind="ExternalInput")
    bw = nc.dram_tensor("bw", [K, D], f32, kind="ExternalInput")
    ws = nc.dram_tensor("ws", [K, C], f32, kind="ExternalInput")
    eye = nc.dram_tensor("eye", [2 * C, C], f16, kind="ExternalInput")
    out = nc.dram_tensor("out", [NT * T, C], f32, kind="ExternalOutput")

    with tile.TileContext(nc) as tc:
        with (
            tc.tile_pool(name="const", bufs=1) as cpool,
            tc.tile_pool(name="idx", bufs=3) as ipool,
            tc.tile_pool(name="gath", bufs=2) as gpool,
            tc.tile_pool(name="uv", bufs=2) as uvpool,
            tc.tile_pool(name="ep", bufs=3) as epool,
            tc.tile_pool(name="ps", bufs=3, space="PSUM") as pspool,
        ):
            # ---- prologue: M' = [W.T @ ws ; I5 ; I5 ; 0] as fp16 [128, C] ----
            w_sb = cpool.tile([K, D], f32)
            nc.sync.dma_start(w_sb[:], bw[:])
            ws_sb = cpool.tile([K, C], f32)
            nc.sync.dma_start(ws_sb[:], ws[:])
            mpsum = pspool.tile([D, C], f32)
            nc.tensor.matmul(out=mpsum[:], lhsT=w_sb[:], rhs=ws_sb[:], start=True, stop=True)
            m16 = cpool.tile([128, C], f16)
            nc.gpsimd.memset(m16[:], 0)
            nc.vector.tensor_copy(out=m16[0:D, :], in_=mpsum[:])
            nc.sync.dma_start(m16[D:D + 2 * C, :], eye[:])

            # ---- main loop over tiles ----
            for t, (su, sv) in enumerate(schedule):
                iu = ipool.tile([128, T // 16], i16, tag="iu")
                nc.sync.dma_start(iu[:], uidx[t, :, :])
                iv = ipool.tile([128, T // 16], i16, tag="iv")
                nc.sync.dma_start(iv[:], vidx[t, :, :])

                ut = gpool.tile([128, T], f16, tag="ut")
                nc.gpsimd.dma_gather(
                    ut[:].rearrange("p (o t) -> p o t", o=1),
                    u16[su * SEG:(su + 1) * SEG, :],
                    iu[:, :], T, T, EL, elem_step=EL, transpose=True, single_packet=False,
                )
                vt = gpool.tile([128, T], f16, tag="vt")
                nc.gpsimd.dma_gather(
                    vt[:].rearrange("p (o t) -> p o t", o=1),
                    v16[sv * SEG:(sv + 1) * SEG, :],
                    iv[:, :], T, T, EL, elem_step=EL, transpose=True, single_packet=False,
                )

                uv = uvpool.tile([128, T], f16, tag="uv")
                nc.vector.tensor_mul(out=uv[:], in0=ut[:], in1=vt[:])

                lp = pspool.tile([128, CH * C], f32, tag="lp")
                for j in range(CH):
                    nc.tensor.matmul(
                        out=lp[:, j * C:(j + 1) * C],
                        lhsT=uv[:, j * 128:(j + 1) * 128],
                        rhs=m16[:, :],
                        start=True, stop=True,
                    )

                ex = epool.tile([128, CH * C], f32, tag="ex")
                nc.scalar.activation(ex[:], lp[:], mybir.ActivationFunctionType.Exp)
                den = epool.tile([128, CH], f32, tag="den")
                nc.vector.tensor_reduce(
                    out=den[:],
                    in_=ex[:].rearrange("p (n c) -> p n c", c=C),
                    axis=mybir.AxisListType.X,
                    op=mybir.AluOpType.add,
                )
                rec = epool.tile([128, CH], f32, tag="rec")
                nc.vector.reciprocal(rec[:], den[:])
                ob = epool.tile([128, CH * C], f32, tag="ob")
                nc.vector.tensor_mul(
                    out=ob[:].rearrange("p (n c) -> p n c", c=C),
                    in0=ex[:].rearrange("p (n c) -> p n c", c=C),
                    in1=rec[:].rearrange("p (n o) -> p n o", o=1).to_broadcast([128, CH, C]),
                )
                nc.sync.dma_start(
                    out[t * T:(t + 1) * T, :].rearrange("(q n) c -> q (n c)", q=128),
                    ob[:],
                )

    nc.compile()
    return nc


def _get_program(schedule):
    key = tuple(schedule)
    if key not in _PROG_CACHE:
        _PROG_CACHE[key] = _build_program(key)
    return _PROG_CACHE[key]


def _prepare(u_feat, v_feat, u_indices, v_indices, basis_weights,
             weights_scalars, user_bias, item_bias):
    u_feat = np.asarray(u_feat, dtype=np.float32)
    v_feat = np.asarray(v_feat, dtype=np.float32)
    u_indices = np.asarray(u_indices, dtype=np.int32)
    v_indices = np.asarray(v_indices, dtype=np.int32)
    basis_weights = np.asarray(basis_weights, dtype=np.float32)
    weights_scalars = np.asarray(weights_scalars, dtype=np.float32)
    user_bias = np.asarray(user_bias, dtype=np.float32)
    item_bias = np.asarray(item_bias, dtype=np.float32)

    E = u_indices.shape[0]
    assert E % CORES == 0
    epc = E // CORES

    # ---- augmented fp16 tables (features + bias folding), 256 B rows ----
    u16 = np.zeros((USEG * SEG, EL), np.float16)
    u16[:NU, :D] = u_feat.astype(np.float16)
    u16[:NU, D:D + C] = user_bias.astype(np.float16)
    u16[:NU, D + C:D + 2 * C] = np.float16(1.0)
    v16 = np.zeros((VSEG * SEG, EL), np.float16)
    v16[:NI, :D] = v_feat.astype(np.float16)
    v16[:NI, D:D + C] = np.float16(1.0)
    v16[:NI, D + C:D + 2 * C] = item_bias.astype(np.float16)
    eye = np.zeros((2 * C, C), np.float16)
    eye[:C] = np.eye(C, dtype=np.float16)
    eye[C:] = np.eye(C, dtype=np.float16)

    # ---- bucket edges per core by (u_segment, v_segment) ----
    NB = USEG * VSEG
    keys = ((u_indices >> 15) * VSEG + (v_indices >> 15)).astype(np.int64)
    orders, counts = [], np.zeros((CORES, NB), np.int64)
    for c in range(CORES):
        kc = keys[c * epc:(c + 1) * epc]
        orders.append(np.argsort(kc, kind="stable"))
        counts[c] = np.bincount(kc, minlength=NB)
    tiles_b = -(-counts.max(axis=0) // T)  # ceil; empty buckets get 0 tiles
    schedule = []
    for b in range(NB):
        schedule += [(b // VSEG, b % VSEG)] * int(tiles_b[b])
    NT = len(schedule)
    ETOT = NT * T
    cap_base = np.concatenate([[0], np.cumsum(tiles_b * T)])

    in_maps, place = [], []
    for c in range(CORES):
        o = orders[c]
        ug = u_indices[c * epc:(c + 1) * epc]
        vg = v_indices[c * epc:(c + 1) * epc]
        u_loc = np.zeros(ETOT, np.int16)
        v_loc = np.zeros(ETOT, np.int16)
        pos = np.empty(epc, np.int64)
        start = 0
        for b in range(NB):
            n = int(counts[c, b])
            sel = o[start:start + n]
            pb = cap_base[b] + np.arange(n)
            u_loc[pb] = (ug[sel] & (SEG - 1)).astype(np.int16)
            v_loc[pb] = (vg[sel] & (SEG - 1)).astype(np.int16)
            pos[start:start + n] = pb
            start += n
        # gather slot i = j*128 + q  <->  padded position q*CH + j
        def to_wrapped(loc):
            g = loc.reshape(NT, 128, CH).transpose(0, 2, 1).reshape(NT, T)
            w = g.reshape(NT, T // 16, 16).transpose(0, 2, 1)  # [NT, 16, T//16]
            w = np.broadcast_to(w[:, None], (NT, 8, 16, T // 16))
            return np.ascontiguousarray(w.reshape(NT, 128, T // 16))
        in_maps.append({
            "u16": u16, "v16": v16,
            "uidx": to_wrapped(u_loc), "vidx": to_wrapped(v_loc),
            "bw": basis_weights, "ws": weights_scalars, "eye": eye,
        })
        place.append((pos, o))

    return schedule, in_maps, place, E, epc


def _unshard(results, place, E, epc):
    out = np.empty((E, C), np.float32)
    for c in range(CORES):
        oc = results[c]["out"]
        pos, o = place[c]
        out[c * epc + o] = oc[pos]
    return out


def kernel(u_feat, v_feat, u_indices, v_indices, basis_weights,
           weights_scalars, user_bias, item_bias):
    import os
    # The bass kernel executes through the axon PJRT backend; don't let a
    # CPU-pinned JAX_PLATFORMS (sometimes set for running jax references)
    # hide the NeuronCore devices.
    if os.environ.get("JAX_PLATFORMS") and "axon" not in os.environ["JAX_PLATFORMS"]:
        os.environ["JAX_PLATFORMS"] = ""
    from concourse.bass_utils import run_bass_kernel_spmd

    schedule, in_maps, place, E, epc = _prepare(
        u_feat, v_feat, u_indices, v_indices, basis_weights,
        weights_scalars, user_bias, item_bias)
    nc = _get_program(tuple(schedule))
    res = run_bass_kernel_spmd(nc, in_maps, core_ids=list(range(CORES)))
    global LAST_RESULT
    LAST_RESULT = res
    return _unshard([r for r in res.results], place, E, epc)


# revision 14
# speedup vs baseline: 2.1641x; 2.1641x over previous
"""Bass/Trainium2 kernel for nn_BilinearMixture (gnn_message_passing).

out = softmax(einsum('ed,kd,ed->ek', u_feat[u_idx], basis_weights, v_feat[v_idx])
              @ weights_scalars + user_bias[u_idx] + item_bias[v_idx])

Strategy (8 NeuronCores, SPMD, edges sharded across cores):
  * Fold the K x D x C basis projection into one M = basis_weights.T @ weights_scalars
    [D, C] matrix (computed on device), so per edge: logits = (u .* v) @ M.
  * Biases are folded into the gather via row augmentation:
      u' = [u, user_bias, 1s, 0...],  v' = [v, 1s, item_bias, 0...],  M' = [M; I; I; 0]
    so logits = sum_d u'_d v'_d M'[d, :] needs no separate bias gathers.
  * Tables are converted to fp16 and padded to 128-element (256 B) rows; edge rows are
    fetched with dma_gather(transpose=True), which lands gathered rows with the feature
    dim on SBUF partitions -- no on-chip transposes needed before the PE matmul.
  * dma_gather indices are int16 (<= 32767), so tables are split into 32768-row
    segments and edges are bucketed host-side by (u_seg, v_seg); each 4096-edge tile
    draws from a single bucket. The bucket permutation is undone host-side.
  * Softmax: logits are tiny (|logit| < ~2), so exp/sum without max subtraction.
"""

import numpy as np

NU, NI, D, E0, K, C = 100000, 50000, 64, 2000000, 3, 5
CORES = 8
SEG = 32768
EL = 128            # fp16 elements per padded table row (256 B)
T = 4096            # edges per tile
CH = T // 128       # matmul chunks (and per-partition edge count) per tile
USEG = (NU + SEG - 1) // SEG   # 4
VSEG = (NI + SEG - 1) // SEG   # 2

_PROG_CACHE = {}


def _build_program(schedule):
    import concourse.bacc as bacc
    import concourse.mybir as mybir
    import concourse.tile as tile
    from concourse.masks import make_identity

    NT = len(schedule)
    f16, f32, i16 = mybir.dt.float16, mybir.dt.float32, mybir.dt.int16

    nc = bacc.Bacc("TRN2", target_bir_lowering=False, debug=False, num_devices=CORES,
                   num_swdge_queues=4)
    u16 = nc.dram_tensor("u16", [USEG * SEG, EL], f16, kind="ExternalInput")
    v16 = nc.dram_tensor("v16", [VSEG * SEG, EL], f16, kind="ExternalInput")
    uidx = nc.dram_tensor("uidx", [NT, 128, T // 16], i16, kind="ExternalInput")
    vidx = nc.dram_tensor("vidx", [NT, 128, T // 16], i16, kind="ExternalInput")
    bw = nc.dram_tensor("bw", [K, D], f32, kind="ExternalInput")
    ws = nc.dram_tensor("ws", [K, C], f32, kind="ExternalInput")
    eye = nc.dram_tensor("eye", [2 * C, C], f16, kind="ExternalInput")
    out = nc.dram_tensor("out", [NT * T, C], f32, kind="ExternalOutput")

    with tile.TileContext(nc) as tc:
        with (
            tc.tile_pool(name="const", bufs=1) as cpool,
            tc.tile_pool(name="idx", bufs=3) as ipool,
            tc.tile_pool(name="gath", bufs=2) as gpool,
            tc.tile_pool(name="uv", bufs=2) as uvpool,
            tc.tile_pool(name="ep", bufs=3) as epool,
            tc.tile_pool(name="uvt", bufs=4) as uvtpool,
            tc.tile_pool(name="ps", bufs=2, space="PSUM") as pspool,
            tc.tile_pool(name="tps", bufs=3, space="PSUM") as tppool,
        ):
            # ---- prologue: M' = [W.T @ ws ; I5 ; I5 ; 0] as fp16 [128, C] ----
            w_sb = cpool.tile([K, D], f32)
            nc.sync.dma_start(w_sb[:], bw[:])
            ws_sb = cpool.tile([K, C], f32)
            nc.sync.dma_start(ws_sb[:], ws[:])
            mpsum = pspool.tile([D, C], f32)
            nc.tensor.matmul(out=mpsum[:], lhsT=w_sb[:], rhs=ws_sb[:], start=True, stop=True)
            m16 = cpool.tile([128, C], f16)
            nc.gpsimd.memset(m16[:], 0)
            nc.vector.tensor_copy(out=m16[0:D, :], in_=mpsum[:])
            nc.sync.dma_start(m16[D:D + 2 * C, :], eye[:])
            ident = cpool.tile([128, 128], f16)
            make_identity(nc, ident[:])

            # ---- main loop over tiles ----
            for t, (su, sv) in enumerate(schedule):
                iu = ipool.tile([128, T // 16], i16, tag="iu")
                nc.sync.dma_start(iu[:], uidx[t, :, :])
                iv = ipool.tile([128, T // 16], i16, tag="iv")
                nc.sync.dma_start(iv[:], vidx[t, :, :])

                ut = gpool.tile([128, CH * EL], f16, tag="ut")
                nc.gpsimd.dma_gather(
                    ut[:].rearrange("p (c e) -> p c e", e=EL),
                    u16[su * SEG:(su + 1) * SEG, :],
                    iu[:, :], T, T, EL, elem_step=EL, transpose=False,
                    single_packet=False, queue_num=(2 * t) % 4,
                )
                vt = gpool.tile([128, CH * EL], f16, tag="vt")
                nc.gpsimd.dma_gather(
                    vt[:].rearrange("p (c e) -> p c e", e=EL),
                    v16[sv * SEG:(sv + 1) * SEG, :],
                    iv[:, :], T, T, EL, elem_step=EL, transpose=False,
                    single_packet=False, queue_num=(2 * t + 1) % 4,
                )

                uv = uvpool.tile([128, CH * EL], f16, tag="uv")
                nc.vector.tensor_mul(out=uv[:], in0=ut[:], in1=vt[:])

                lp = pspool.tile([128, CH * C], f32, tag="lp")
                for j in range(CH):
                    tp = tppool.tile([128, 128], f16, tag="tp")
                    nc.tensor.transpose(
                        out=tp[:], in_=uv[:, j * EL:(j + 1) * EL], identity=ident[:],
                    )
                    uvt = uvtpool.tile([128, 128], f16, tag="uvt")
                    nc.vector.tensor_copy(out=uvt[:], in_=tp[:])
                    nc.tensor.matmul(
                        out=lp[:, j * C:(j + 1) * C],
                        lhsT=uvt[:],
                        rhs=m16[:, :],
                        start=True, stop=True,
                    )

                ex = epool.tile([128, CH * C], f32, tag="ex")
                nc.scalar.activation(ex[:], lp[:], mybir.ActivationFunctionType.Exp)
                den = epool.tile([128, CH], f32, tag="den")
                nc.vector.tensor_reduce(
                    out=den[:],
                    in_=ex[:].rearrange("p (n c) -> p n c", c=C),
                    axis=mybir.AxisListType.X,
                    op=mybir.AluOpType.add,
                )
                rec = epool.tile([128, CH], f32, tag="rec")
                nc.vector.reciprocal(rec[:], den[:])
                ob = epool.tile([128, CH * C], f32, tag="ob")
                nc.vector.tensor_mul(
                    out=ob[:].rearrange("p (n c) -> p n c", c=C),
                    in0=ex[:].rearrange("p (n c) -> p n c", c=C),
                    in1=rec[:].rearrange("p (n o) -> p n o", o=1).to_broadcast([128, CH, C]),
                )
                nc.sync.dma_start(
                    out[t * T:(t + 1) * T, :].rearrange("(q n) c -> q (n c)", q=128),
                    ob[:],
                )

    nc.compile()
    return nc


def _get_program(schedule):
    key = tuple(schedule)
    if key not in _PROG_CACHE:
        _PROG_CACHE[key] = _build_program(key)
    return _PROG_CACHE[key]


def _prepare(u_feat, v_feat, u_indices, v_indices, basis_weights,
             weights_scalars, user_bias, item_bias):
    u_feat = np.asarray(u_feat, dtype=np.float32)
    v_feat = np.asarray(v_feat, dtype=np.float32)
    u_indices = np.asarray(u_indices, dtype=np.int32)
    v_indices = np.asarray(v_indices, dtype=np.int32)
    basis_weights = np.asarray(basis_weights, dtype=np.float32)
    weights_scalars = np.asarray(weights_scalars, dtype=np.float32)
    user_bias = np.asarray(user_bias, dtype=np.float32)
    item_bias = np.asarray(item_bias, dtype=np.float32)

    E = u_indices.shape[0]
    assert E % CORES == 0
    epc = E // CORES

    # ---- augmented fp16 tables (features + bias folding), 256 B rows ----
    u16 = np.zeros((USEG * SEG, EL), np.float16)
    u16[:NU, :D] = u_feat.astype(np.float16)
    u16[:NU, D:D + C] = user_bias.astype(np.float16)
    u16[:NU, D + C:D + 2 * C] = np.float16(1.0)
    v16 = np.zeros((VSEG * SEG, EL), np.float16)
    v16[:NI, :D] = v_feat.astype(np.float16)
    v16[:NI, D:D + C] = np.float16(1.0)
    v16[:NI, D + C:D + 2 * C] = item_bias.astype(np.float16)
    eye = np.zeros((2 * C, C), np.float16)
    eye[:C] = np.eye(C, dtype=np.float16)
    eye[C:] = np.eye(C, dtype=np.float16)

    # ---- bucket edges per core by (u_segment, v_segment) ----
    NB = USEG * VSEG
    keys = ((u_indices >> 15) * VSEG + (v_indices >> 15)).astype(np.int64)
    orders, counts = [], np.zeros((CORES, NB), np.int64)
    for c in range(CORES):
        kc = keys[c * epc:(c + 1) * epc]
        orders.append(np.argsort(kc, kind="stable"))
        counts[c] = np.bincount(kc, minlength=NB)
    tiles_b = -(-counts.max(axis=0) // T)  # ceil; empty buckets get 0 tiles
    schedule = []
    for b in range(NB):
        schedule += [(b // VSEG, b % VSEG)] * int(tiles_b[b])
    NT = len(schedule)
    ETOT = NT * T
    cap_base = np.concatenate([[0], np.cumsum(tiles_b * T)])

    in_maps, place = [], []
    for c in range(CORES):
        o = orders[c]
        ug = u_indices[c * epc:(c + 1) * epc]
        vg = v_indices[c * epc:(c + 1) * epc]
        u_loc = np.zeros(ETOT, np.int16)
        v_loc = np.zeros(ETOT, np.int16)
        pos = np.empty(epc, np.int64)
        start = 0
        for b in range(NB):
            n = int(counts[c, b])
            sel = o[start:start + n]
            pb = cap_base[b] + np.arange(n)
            u_loc[pb] = (ug[sel] & (SEG - 1)).astype(np.int16)
            v_loc[pb] = (vg[sel] & (SEG - 1)).astype(np.int16)
            pos[start:start + n] = pb
            start += n
        # gather slot i = j*128 + q  <->  padded position q*CH + j
        def to_wrapped(loc):
            g = loc.reshape(NT, 128, CH).transpose(0, 2, 1).reshape(NT, T)
            w = g.reshape(NT, T // 16, 16).transpose(0, 2, 1)  # [NT, 16, T//16]
            w = np.broadcast_to(w[:, None], (NT, 8, 16, T // 16))
            return np.ascontiguousarray(w.reshape(NT, 128, T // 16))
        in_maps.append({
            "u16": u16, "v16": v16,
            "uidx": to_wrapped(u_loc), "vidx": to_wrapped(v_loc),
            "bw": basis_weights, "ws": weights_scalars, "eye": eye,
        })
        place.append((pos, o))

    return schedule, in_maps, place, E, epc


def _unshard(results, place, E, epc):
    out = np.empty((E, C), np.float32)
    for c in range(CORES):
        oc = results[c]["out"]
        pos, o = place[c]
        out[c * epc + o] = oc[pos]
    return out


def kernel(u_feat, v_feat, u_indices, v_indices, basis_weights,
           weights_scalars, user_bias, item_bias):
    import os
    # The bass kernel executes through the axon PJRT backend; don't let a
    # CPU-pinned JAX_PLATFORMS (sometimes set for running jax references)
    # hide the NeuronCore devices.
    if os.environ.get("JAX_PLATFORMS") and "axon" not in os.environ["JAX_PLATFORMS"]:
        os.environ["JAX_PLATFORMS"] = ""
    from concourse.bass_utils import run_bass_kernel_spmd

    schedule, in_maps, place, E, epc = _prepare(
        u_feat, v_feat, u_indices, v_indices, basis_weights,
        weights_scalars, user_bias, item_bias)
    nc = _get_program(tuple(schedule))
    res = run_bass_kernel_spmd(nc, in_maps, core_ids=list(range(CORES)))
    global LAST_RESULT
    LAST_RESULT = res
    return _unshard([r for r in res.results], place, E, epc)


# revision 16
# speedup vs baseline: 2.1851x; 1.0097x over previous
"""Bass/Trainium2 kernel for nn_BilinearMixture (gnn_message_passing).

out = softmax(einsum('ed,kd,ed->ek', u_feat[u_idx], basis_weights, v_feat[v_idx])
              @ weights_scalars + user_bias[u_idx] + item_bias[v_idx])

Strategy (8 NeuronCores, SPMD, edges sharded across cores):
  * Fold the K x D x C basis projection into one M = basis_weights.T @ weights_scalars
    [D, C] matrix (computed on device), so per edge: logits = (u .* v) @ M.
  * Biases are folded into the gather via row augmentation:
      u' = [u, user_bias, 1s, 0...],  v' = [v, 1s, item_bias, 0...],  M' = [M; I; I; 0]
    so logits = sum_d u'_d v'_d M'[d, :] needs no separate bias gathers.
  * Tables are converted to fp16 and padded to 128-element (256 B) rows; edge rows are
    fetched with dma_gather(transpose=True), which lands gathered rows with the feature
    dim on SBUF partitions -- no on-chip transposes needed before the PE matmul.
  * dma_gather indices are int16 (<= 32767), so tables are split into 32768-row
    segments and edges are bucketed host-side by (u_seg, v_seg); each 4096-edge tile
    draws from a single bucket. The bucket permutation is undone host-side.
  * Softmax: logits are tiny (|logit| < ~2), so exp/sum without max subtraction.
"""

import numpy as np

NU, NI, D, E0, K, C = 100000, 50000, 64, 2000000, 3, 5
CORES = 8
SEG = 32768
EL = 128            # fp16 elements per padded table row (256 B)
T = 4096            # edges per tile
CH = T // 128       # matmul chunks (and per-partition edge count) per tile
USEG = (NU + SEG - 1) // SEG   # 4
VSEG = (NI + SEG - 1) // SEG   # 2

_PROG_CACHE = {}


def _build_program(schedule):
    import concourse.bacc as bacc
    import concourse.mybir as mybir
    import concourse.tile as tile
    from concourse.masks import make_identity

    NT = len(schedule)
    f16, f32, i16 = mybir.dt.float16, mybir.dt.float32, mybir.dt.int16

    nc = bacc.Bacc("TRN2", target_bir_lowering=False, debug=False, num_devices=CORES,
                   num_swdge_queues=4)
    u16 = nc.dram_tensor("u16", [USEG * SEG, EL], f16, kind="ExternalInput")
    v16 = nc.dram_tensor("v16", [VSEG * SEG, EL], f16, kind="ExternalInput")
    uidx = nc.dram_tensor("uidx", [NT, 128, T // 16], i16, kind="ExternalInput")
    vidx = nc.dram_tensor("vidx", [NT, 128, T // 16], i16, kind="ExternalInput")
    bw = nc.dram_tensor("bw", [K, D], f32, kind="ExternalInput")
    ws = nc.dram_tensor("ws", [K, C], f32, kind="ExternalInput")
    eye = nc.dram_tensor("eye", [2 * C, C], f16, kind="ExternalInput")
    out = nc.dram_tensor("out", [NT * T, C], f32, kind="ExternalOutput")

    with tile.TileContext(nc) as tc:
        with (
            tc.tile_pool(name="const", bufs=1) as cpool,
            tc.tile_pool(name="idx", bufs=4) as ipool,
            tc.tile_pool(name="gath", bufs=3) as gpool,
            tc.tile_pool(name="uv", bufs=2) as uvpool,
            tc.tile_pool(name="ep", bufs=3) as epool,
            tc.tile_pool(name="uvt", bufs=2) as uvtpool,
            tc.tile_pool(name="ps", bufs=2, space="PSUM") as pspool,
            tc.tile_pool(name="tps", bufs=2, space="PSUM") as tppool,
        ):
            # ---- prologue: M' = [W.T @ ws ; I5 ; I5 ; 0] as fp16 [128, C] ----
            w_sb = cpool.tile([K, D], f32)
            nc.sync.dma_start(w_sb[:], bw[:])
            ws_sb = cpool.tile([K, C], f32)
            nc.sync.dma_start(ws_sb[:], ws[:])
            mpsum = pspool.tile([D, C], f32)
            nc.tensor.matmul(out=mpsum[:], lhsT=w_sb[:], rhs=ws_sb[:], start=True, stop=True)
            m16 = cpool.tile([128, C], f16)
            nc.gpsimd.memset(m16[:], 0)
            nc.vector.tensor_copy(out=m16[0:D, :], in_=mpsum[:])
            nc.sync.dma_start(m16[D:D + 2 * C, :], eye[:])
            ident = cpool.tile([128, 128], f16)
            make_identity(nc, ident[:])

            # ---- main loop over tiles ----
            for t, (su, sv) in enumerate(schedule):
                iu = ipool.tile([128, T // 16], i16, tag="iu")
                nc.sync.dma_start(iu[:], uidx[t, :, :])
                iv = ipool.tile([128, T // 16], i16, tag="iv")
                nc.sync.dma_start(iv[:], vidx[t, :, :])

                ut = gpool.tile([128, CH * EL], f16, tag="ut")
                nc.gpsimd.dma_gather(
                    ut[:].rearrange("p (c e) -> p c e", e=EL),
                    u16[su * SEG:(su + 1) * SEG, :],
                    iu[:, :], T, T, EL, elem_step=EL, transpose=False,
                    single_packet=False, queue_num=(2 * t) % 4,
                )
                vt = gpool.tile([128, CH * EL], f16, tag="vt")
                nc.gpsimd.dma_gather(
                    vt[:].rearrange("p (c e) -> p c e", e=EL),
                    v16[sv * SEG:(sv + 1) * SEG, :],
                    iv[:, :], T, T, EL, elem_step=EL, transpose=False,
                    single_packet=False, queue_num=(2 * t + 1) % 4,
                )

                uv = uvpool.tile([128, CH * EL], f16, tag="uv")
                nc.vector.tensor_mul(out=uv[:], in0=ut[:], in1=vt[:])

                # Phase-separated: 8 transposes into one PSUM bank, one batched
                # PSUM->SBUF copy, then 8 back-to-back projection matmuls (lets
                # the PE background weight buffer pipeline the LDWEIGHTS).
                GRP = 8
                lp = pspool.tile([128, CH * C], f32, tag="lp")
                for g in range(CH // GRP):
                    tp = tppool.tile([128, GRP * 128], f16, tag="tp")
                    for k in range(GRP):
                        j = g * GRP + k
                        nc.tensor.transpose(
                            out=tp[:, k * 128:(k + 1) * 128],
                            in_=uv[:, j * EL:(j + 1) * EL],
                            identity=ident[:],
                        )
                    uvt = uvtpool.tile([128, GRP * 128], f16, tag="uvt")
                    nc.vector.tensor_copy(out=uvt[:], in_=tp[:])
                    for k in range(GRP):
                        j = g * GRP + k
                        nc.tensor.matmul(
                            out=lp[:, j * C:(j + 1) * C],
                            lhsT=uvt[:, k * 128:(k + 1) * 128],
                            rhs=m16[:, :],
                            start=True, stop=True,
                        )

                ex = epool.tile([128, CH * C], f32, tag="ex")
                nc.scalar.activation(ex[:], lp[:], mybir.ActivationFunctionType.Exp)
                den = epool.tile([128, CH], f32, tag="den")
                nc.vector.tensor_reduce(
                    out=den[:],
                    in_=ex[:].rearrange("p (n c) -> p n c", c=C),
                    axis=mybir.AxisListType.X,
                    op=mybir.AluOpType.add,
                )
                rec = epool.tile([128, CH], f32, tag="rec")
                nc.vector.reciprocal(rec[:], den[:])
                ob = epool.tile([128, CH * C], f32, tag="ob")
                nc.vector.tensor_mul(
                    out=ob[:].rearrange("p (n c) -> p n c", c=C),
                    in0=ex[:].rearrange("p (n c) -> p n c", c=C),
                    in1=rec[:].rearrange("p (n o) -> p n o", o=1).to_broadcast([128, CH, C]),
                )
                nc.sync.dma_start(
                    out[t * T:(t + 1) * T, :].rearrange("(q n) c -> q (n c)", q=128),
                    ob[:],
                )

    nc.compile()
    return nc


def _get_program(schedule):
    key = tuple(schedule)
    if key not in _PROG_CACHE:
        _PROG_CACHE[key] = _build_program(key)
    return _PROG_CACHE[key]


def _prepare(u_feat, v_feat, u_indices, v_indices, basis_weights,
             weights_scalars, user_bias, item_bias):
    u_feat = np.asarray(u_feat, dtype=np.float32)
    v_feat = np.asarray(v_feat, dtype=np.float32)
    u_indices = np.asarray(u_indices, dtype=np.int32)
    v_indices = np.asarray(v_indices, dtype=np.int32)
    basis_weights = np.asarray(basis_weights, dtype=np.float32)
    weights_scalars = np.asarray(weights_scalars, dtype=np.float32)
    user_bias = np.asarray(user_bias, dtype=np.float32)
    item_bias = np.asarray(item_bias, dtype=np.float32)

    E = u_indices.shape[0]
    assert E % CORES == 0
    epc = E // CORES

    # ---- augmented fp16 tables (features + bias folding), 256 B rows ----
    u16 = np.zeros((USEG * SEG, EL), np.float16)
    u16[:NU, :D] = u_feat.astype(np.float16)
    u16[:NU, D:D + C] = user_bias.astype(np.float16)
    u16[:NU, D + C:D + 2 * C] = np.float16(1.0)
    v16 = np.zeros((VSEG * SEG, EL), np.float16)
    v16[:NI, :D] = v_feat.astype(np.float16)
    v16[:NI, D:D + C] = np.float16(1.0)
    v16[:NI, D + C:D + 2 * C] = item_bias.astype(np.float16)
    eye = np.zeros((2 * C, C), np.float16)
    eye[:C] = np.eye(C, dtype=np.float16)
    eye[C:] = np.eye(C, dtype=np.float16)

    # ---- bucket edges per core by (u_segment, v_segment) ----
    NB = USEG * VSEG
    keys = ((u_indices >> 15) * VSEG + (v_indices >> 15)).astype(np.int64)
    orders, counts = [], np.zeros((CORES, NB), np.int64)
    for c in range(CORES):
        kc = keys[c * epc:(c + 1) * epc]
        orders.append(np.argsort(kc, kind="stable"))
        counts[c] = np.bincount(kc, minlength=NB)
    tiles_b = -(-counts.max(axis=0) // T)  # ceil; empty buckets get 0 tiles
    schedule = []
    for b in range(NB):
        schedule += [(b // VSEG, b % VSEG)] * int(tiles_b[b])
    NT = len(schedule)
    ETOT = NT * T
    cap_base = np.concatenate([[0], np.cumsum(tiles_b * T)])

    in_maps, place = [], []
    for c in range(CORES):
        o = orders[c]
        ug = u_indices[c * epc:(c + 1) * epc]
        vg = v_indices[c * epc:(c + 1) * epc]
        u_loc = np.zeros(ETOT, np.int16)
        v_loc = np.zeros(ETOT, np.int16)
        pos = np.empty(epc, np.int64)
        start = 0
        for b in range(NB):
            n = int(counts[c, b])
            sel = o[start:start + n]
            pb = cap_base[b] + np.arange(n)
            u_loc[pb] = (ug[sel] & (SEG - 1)).astype(np.int16)
            v_loc[pb] = (vg[sel] & (SEG - 1)).astype(np.int16)
            pos[start:start + n] = pb
            start += n
        # gather slot i = j*128 + q  <->  padded position q*CH + j
        def to_wrapped(loc):
            g = loc.reshape(NT, 128, CH).transpose(0, 2, 1).reshape(NT, T)
            w = g.reshape(NT, T // 16, 16).transpose(0, 2, 1)  # [NT, 16, T//16]
            w = np.broadcast_to(w[:, None], (NT, 8, 16, T // 16))
            return np.ascontiguousarray(w.reshape(NT, 128, T // 16))
        in_maps.append({
            "u16": u16, "v16": v16,
            "uidx": to_wrapped(u_loc), "vidx": to_wrapped(v_loc),
            "bw": basis_weights, "ws": weights_scalars, "eye": eye,
        })
        place.append((pos, o))

    return schedule, in_maps, place, E, epc


def _unshard(results, place, E, epc):
    out = np.empty((E, C), np.float32)
    for c in range(CORES):
        oc = results[c]["out"]
        pos, o = place[c]
        out[c * epc + o] = oc[pos]
    return out


def kernel(u_feat, v_feat, u_indices, v_indices, basis_weights,
           weights_scalars, user_bias, item_bias):
    import os
    # The bass kernel executes through the axon PJRT backend; don't let a
    # CPU-pinned JAX_PLATFORMS (sometimes set for running jax references)
    # hide the NeuronCore devices.
    if os.environ.get("JAX_PLATFORMS") and "axon" not in os.environ["JAX_PLATFORMS"]:
        os.environ["JAX_PLATFORMS"] = ""
    from concourse.bass_utils import run_bass_kernel_spmd

    schedule, in_maps, place, E, epc = _prepare(
        u_feat, v_feat, u_indices, v_indices, basis_weights,
        weights_scalars, user_bias, item_bias)
    nc = _get_program(tuple(schedule))
    res = run_bass_kernel_spmd(nc, in_maps, core_ids=list(range(CORES)))
    global LAST_RESULT
    LAST_RESULT = res
    return _unshard([r for r in res.results], place, E, epc)
